# revision 27
# baseline (speedup 1.0000x reference)
"""KMeans cluster kernel for 8-core TRN2 — builder + host wrapper.

Data-parallel over samples: each of the 8 cores owns 8192 rows of x.
Per epoch: dist = x @ cent.T via PE (fp32, exact), argmin via DVE
min-reduce + is_equal one-hot (written as bf16 — 0/1 exact),
per-centroid sums+counts via three bf16 one-hot matmuls accumulated in
PSUM (xa is split exactly into three bf16 terms hi/mid/lo on the host:
24-bit mantissa = 3x8-bit chunks, zero residual; bf16 matmuls stream
1 col/cycle vs fp32's 4), AllReduce across cores, centroid mean update
+ PE transpose. Epoch 10 extracts indices only (scalar_tensor_tensor
accum trick). Sums stage trails the dist stage by SUMS_LAG chunks so
the in-order PE never stalls on the DVE argmin chain.
"""

import numpy as np
import ml_dtypes
import concourse.bass as bass
import concourse.bacc as bacc
import concourse.tile as tile
import concourse.mybir as mybir
from concourse import bass_utils

N_CORES = 8
N = 65536
D = 256
DP = D + 2                # ones col (counts) + zero pad
K = 512
NSH = N // N_CORES        # rows per core
NCH = NSH // 128          # chunks of 128 rows
EPOCHS = 10

F32 = mybir.dt.float32
BF16 = mybir.dt.bfloat16
I32 = mybir.dt.int32
AX = mybir.AxisListType.X
OP = mybir.AluOpType

SUMS_LAG = 2              # chunks the sums stage trails the dist stage


def build(trials=1):
    nc = bacc.Bacc("TRN2", target_bir_lowering=False, debug=False,
                   num_devices=N_CORES)
    xa3 = [nc.dram_tensor(f"xa{s}", [NSH, DP], BF16, kind="ExternalInput").ap()
           for s in range(3)]
    xt = nc.dram_tensor("xt", [D, NSH], F32, kind="ExternalInput").ap()
    c0t = nc.dram_tensor("c0t", [D, K], F32, kind="ExternalInput").ap()
    c0 = nc.dram_tensor("c0", [K, D], F32, kind="ExternalInput").ap()
    iotaf = nc.dram_tensor("iotaf", [128, K], F32, kind="ExternalInput").ap()
    ident = nc.dram_tensor("ident", [128, 128], F32, kind="ExternalInput").ap()
    idx_out = nc.dram_tensor("idx_out", [NCH, 128], I32, kind="ExternalOutput").ap()

    snd = [nc.dram_tensor(f"snd{e}", [K, DP], F32, kind="Internal").ap()
           for e in range((EPOCHS - 1) * trials)]
    rcv = [nc.dram_tensor(f"rcv{e}", [K, DP], F32, kind="Internal",
                          addr_space="Shared").ap()
           for e in range((EPOCHS - 1) * trials)]
    warm_s = nc.dram_tensor("warm_s", [K, DP], F32, kind="Internal").ap()
    warm_r = nc.dram_tensor("warm_r", [K, DP], F32, kind="Internal",
                            addr_space="Shared").ap()
    rg = [list(range(N_CORES))]

    with tile.TileContext(nc) as tc:
        with (tc.tile_pool(name="big", bufs=1) as big,
              tc.tile_pool(name="work", bufs=3) as work,
              tc.tile_pool(name="small", bufs=8) as small,
              tc.tile_pool(name="ps", bufs=3, space="PSUM") as psp,
              tc.tile_pool(name="pss", bufs=1, space="PSUM") as pss):
            # initial loads on two parallel HWDGE queues (sync + scalar),
            # ordered by first use so compute starts immediately.
            centT = [big.tile([128, 2, K], F32, name=f"centT{b}") for b in range(2)]
            cent_kd = [big.tile([128, 4, D], F32, name=f"centkd{b}") for b in range(2)]
            for dc in range(2):
                nc.sync.dma_start(centT[0][:, dc, :], c0t[dc * 128:(dc + 1) * 128, :])
            for kc in range(4):
                nc.sync.dma_start(cent_kd[0][:, kc, :], c0[kc * 128:(kc + 1) * 128, :])
            xt_sb = big.tile([128, 2, NSH], F32)
            for j in range(8):
                for dc in range(2):
                    nc.sync.dma_start(
                        xt_sb[:, dc, j * 1024:(j + 1) * 1024],
                        xt[dc * 128:(dc + 1) * 128, j * 1024:(j + 1) * 1024])
            xa_sb = [big.tile([128, NCH, DP], BF16, name=f"xa{s}_sb")
                     for s in range(3)]
            for i in range(NCH):
                for s in range(3):
                    nc.scalar.dma_start(xa_sb[s][:, i, :],
                                        xa3[s][i * 128:(i + 1) * 128, :])
            iota_sb = big.tile([128, K], F32)
            nc.scalar.dma_start(iota_sb[:, :], iotaf[:, :])
            ident_sb = big.tile([128, 128], F32)
            nc.scalar.dma_start(ident_sb[:, :], ident[:, :])
            # warm up the collective stack (full payload size) while the
            # input loads stream in. The send stages the last-loaded xa tile,
            # so this AR doubles as a cross-core load-completion barrier —
            # epoch 0's real AR then doesn't absorb inter-core load skew.
            warm_sb = small.tile([128, 2], F32, tag="warm")
            nc.gpsimd.tensor_copy(warm_sb[:, :], xa_sb[2][:, NCH - 1, 0:2])
            nc.sync.dma_start(warm_s[0:128, 0:2], warm_sb[:, :])
            nc.gpsimd.collective_compute(
                "AllReduce", OP.add, replica_groups=rg,
                ins=[warm_s[:, :].opt()], outs=[warm_r[:, :].opt()])

            def dist_stage(e, i, cur, last):
                # dist in quarter-K column blocks so epoch e's PE work can
                # begin as soon as the first updated centroid quarter lands.
                dist_ps = psp.tile([128, K], F32, tag="dist", name=f"dist_{e}_{i}")
                for q in range(2):
                    cols = slice(q * 256, (q + 1) * 256)
                    for dc in range(2):
                        nc.tensor.matmul(dist_ps[:, cols],
                                         xt_sb[:, dc, i * 128:(i + 1) * 128],
                                         cur[:, dc, cols],
                                         start=(dc == 0), stop=(dc == 1))
                minv = small.tile([128, 1], F32, tag="minv", name=f"minv_{e}_{i}")
                nc.vector.tensor_reduce(minv[:, :], dist_ps[:, :], axis=AX, op=OP.min)
                if not last:
                    A = work.tile([128, K], BF16, tag="A", name=f"A_{e}_{i}")
                    nc.vector.tensor_scalar(A[:, :], dist_ps[:, :], minv[:, :],
                                            None, OP.is_equal)
                    return A
                junk = work.tile([128, K], BF16, tag="junk", name=f"junk_{i}", bufs=2)
                idxf = small.tile([128, 1], F32, tag="idxf", name=f"idxf_{i}")
                nc.vector.scalar_tensor_tensor(junk[:, :], dist_ps[:, :],
                                               minv[:, :], iota_sb[:, :],
                                               OP.is_equal, OP.mult,
                                               accum_out=idxf[:, :])
                idxi = small.tile([128, 1], I32, tag="idxi", name=f"idxi_{i}")
                nc.vector.tensor_copy(idxi[:, :], idxf[:, :])
                nc.sync.dma_start(idx_out[i:i + 1, :], idxi[:, :])
                return None

            def sums_stage(i, A, sums_ps):
                for kc in range(4):
                    for s in range(3):
                        nc.tensor.matmul(sums_ps[kc][:, :],
                                         A[:, kc * 128:(kc + 1) * 128],
                                         xa_sb[s][:, i, :],
                                         start=(i == 0 and s == 0),
                                         stop=(i == NCH - 1 and s == 2))

            for t in range(trials):
              for e_ in range(EPOCHS):
                e = t * EPOCHS + e_
                last = e_ == EPOCHS - 1
                cur = centT[e_ % 2]
                sums_ps = None
                if not last:
                    sums_ps = [pss.tile([128, DP], F32, tag=f"sums{kc}",
                                        name=f"sums_{e}_{kc}") for kc in range(4)]
                pend = []
                for i in range(NCH):
                    A = dist_stage(e, i, cur, last)
                    if not last:
                        pend.append((i, A))
                        if len(pend) > SUMS_LAG:
                            j, Aj = pend.pop(0)
                            sums_stage(j, Aj, sums_ps)
                for j, Aj in pend:
                    sums_stage(j, Aj, sums_ps)
                if last:
                    continue

                ce = t * (EPOCHS - 1) + e_
                sums_sb = work.tile([128, 4, DP], F32, tag="sumssb",
                                    name=f"sumssb_{e}", bufs=1)
                sums_red = work.tile([128, 4, DP], F32, tag="sumsred",
                                     name=f"sumsred_{e}", bufs=1)
                for kc in range(4):
                    nc.vector.tensor_copy(sums_sb[:, kc, :], sums_ps[kc][:, :])
                    eng = nc.sync if kc % 2 == 0 else nc.scalar
                    eng.dma_start(snd[ce][kc * 128:(kc + 1) * 128, :],
                                  sums_sb[:, kc, :])
                nc.gpsimd.collective_compute(
                    "AllReduce", OP.add, replica_groups=rg,
                    ins=[snd[ce][:, :].opt()], outs=[rcv[ce][:, :].opt()])
                for kc in range(4):
                    eng = nc.sync if kc % 2 == 0 else nc.scalar
                    eng.dma_start(sums_red[:, kc, :],
                                  rcv[ce][kc * 128:(kc + 1) * 128, :])

                old_kd = cent_kd[e_ % 2]
                new_kd = cent_kd[(e_ + 1) % 2]
                nxt = centT[(e_ + 1) % 2]
                for kc in range(4):
                    counts = sums_red[:, kc, D:D + 1]
                    safe = small.tile([128, 1], F32, tag="safe", name=f"safe_{e}_{kc}")
                    nc.vector.tensor_scalar(safe[:, :], counts, 1.0, None, OP.max)
                    inv = small.tile([128, 1], F32, tag="inv", name=f"inv_{e}_{kc}")
                    nc.vector.reciprocal(inv[:, :], safe[:, :])
                    mask0 = small.tile([128, 1], F32, tag="mask0", name=f"m0_{e}_{kc}")
                    nc.vector.tensor_scalar(mask0[:, :], counts, 0.0, None, OP.is_equal)
                    oldm = work.tile([128, D], F32, tag="oldm", name=f"oldm_{e}_{kc}",
                                     bufs=2)
                    nc.gpsimd.tensor_scalar(oldm[:, :], old_kd[:, kc, :],
                                            mask0[:, :], None, OP.mult)
                    # new = sums*inv + old*(counts==0), fused on DVE
                    nc.vector.scalar_tensor_tensor(new_kd[:, kc, :],
                                                   sums_red[:, kc, 0:D],
                                                   inv[:, :], oldm[:, :],
                                                   OP.mult, OP.add)
                    for dc in range(2):
                        tp = psp.tile([128, 128], F32, tag="dist",
                                      name=f"tp_{e}_{kc}_{dc}")
                        nc.tensor.transpose(tp[:, :],
                                            new_kd[:, kc, dc * 128:(dc + 1) * 128],
                                            ident_sb[:, :])
                        nc.vector.tensor_copy(nxt[:, dc, kc * 128:(kc + 1) * 128],
                                              tp[:, :])
    nc.compile()
    return nc


_NC_CACHE = {}


def get_nc(trials=1):
    if trials not in _NC_CACHE:
        _NC_CACHE[trials] = build(trials)
    return _NC_CACHE[trials]


def make_in_maps(x):
    bf = ml_dtypes.bfloat16
    x = np.ascontiguousarray(np.asarray(x, dtype=np.float32))
    assert x.shape == (N, D)
    cent0 = x[:K]
    c0t_np = np.ascontiguousarray(cent0.T)
    c0_np = np.ascontiguousarray(cent0)
    iota_np = np.broadcast_to(np.arange(K, dtype=np.float32), (128, K)).copy()
    ident_np = np.eye(128, dtype=np.float32)
    in_maps = []
    for r in range(N_CORES):
        xs = x[r * NSH:(r + 1) * NSH]
        xa_np = np.concatenate([xs, np.ones((NSH, 1), np.float32),
                                np.zeros((NSH, 1), np.float32)], axis=1)
        # exact 3-way bf16 split: xa == xh + xm + xl bit-exactly
        xh = xa_np.astype(bf)
        rr = xa_np - xh.astype(np.float32)
        xm = rr.astype(bf)
        xl = (rr - xm.astype(np.float32)).astype(bf)
        xt_np = np.ascontiguousarray(xs.T)
        in_maps.append({
            "xa0": np.ascontiguousarray(xh),
            "xa1": np.ascontiguousarray(xm),
            "xa2": np.ascontiguousarray(xl),
            "xt": xt_np,
            "c0t": c0t_np,
            "c0": c0_np,
            "iotaf": iota_np,
            "ident": ident_np,
        })
    return in_maps


def kernel(x):
    """Full-input k-means kernel: shards x over 8 TRN2 cores internally."""
    nc = get_nc()
    in_maps = make_in_maps(x)
    res = bass_utils.run_bass_kernel_spmd(nc, in_maps,
                                          core_ids=list(range(N_CORES)))
    idx = np.concatenate([res.results[r]["idx_out"].reshape(-1)
                          for r in range(N_CORES)]).astype(np.int32)
    return idx


# revision 28
# speedup vs baseline: 1.0396x; 1.0396x over previous
"""KMeans cluster kernel for 8-core TRN2 — builder + host wrapper.

Data-parallel over samples: each of the 8 cores owns 8192 rows of x.
Per epoch: dist = x @ cent.T via PE (fp32, exact), argmin via DVE
min-reduce + is_equal one-hot (written as bf16 — 0/1 exact),
per-centroid sums+counts via three bf16 one-hot matmuls accumulated in
PSUM (xa is split exactly into three bf16 terms hi/mid/lo on the host:
24-bit mantissa = 3x8-bit chunks, zero residual; bf16 matmuls stream
1 col/cycle vs fp32's 4), AllReduce across cores, centroid mean update
+ PE transpose. Epoch 10 extracts indices only (scalar_tensor_tensor
accum trick). Sums stage trails the dist stage by SUMS_LAG chunks so
the in-order PE never stalls on the DVE argmin chain.
"""

import numpy as np
import ml_dtypes
import concourse.bass as bass
import concourse.bacc as bacc
import concourse.tile as tile
import concourse.mybir as mybir
from concourse import bass_utils

N_CORES = 8
N = 65536
D = 256
DP = D + 2                # ones col (counts) + zero pad
K = 512
NSH = N // N_CORES        # rows per core
NCH = NSH // 128          # chunks of 128 rows
EPOCHS = 10

F32 = mybir.dt.float32
BF16 = mybir.dt.bfloat16
I32 = mybir.dt.int32
AX = mybir.AxisListType.X
OP = mybir.AluOpType

SUMS_LAG = 2              # chunks the sums stage trails the dist stage


def build(trials=1):
    nc = bacc.Bacc("TRN2", target_bir_lowering=False, debug=False,
                   num_devices=N_CORES)
    xa3 = [nc.dram_tensor(f"xa{s}", [NSH, DP], BF16, kind="ExternalInput").ap()
           for s in range(3)]
    xt = nc.dram_tensor("xt", [D, NSH], F32, kind="ExternalInput").ap()
    c0t = nc.dram_tensor("c0t", [D, K], F32, kind="ExternalInput").ap()
    c0 = nc.dram_tensor("c0", [K, D], F32, kind="ExternalInput").ap()
    iotaf = nc.dram_tensor("iotaf", [128, K], F32, kind="ExternalInput").ap()
    ident = nc.dram_tensor("ident", [128, 128], F32, kind="ExternalInput").ap()
    idx_out = nc.dram_tensor("idx_out", [NCH, 128], I32, kind="ExternalOutput").ap()

    snd = [nc.dram_tensor(f"snd{e}", [K, DP], F32, kind="Internal").ap()
           for e in range((EPOCHS - 1) * trials)]
    rcv = [nc.dram_tensor(f"rcv{e}", [K, DP], F32, kind="Internal",
                          addr_space="Shared").ap()
           for e in range((EPOCHS - 1) * trials)]
    warm_s = nc.dram_tensor("warm_s", [K, DP], F32, kind="Internal").ap()
    warm_r = nc.dram_tensor("warm_r", [K, DP], F32, kind="Internal",
                            addr_space="Shared").ap()
    rg = [list(range(N_CORES))]

    with tile.TileContext(nc) as tc:
        with (tc.tile_pool(name="big", bufs=1) as big,
              tc.tile_pool(name="work", bufs=3) as work,
              tc.tile_pool(name="small", bufs=8) as small,
              tc.tile_pool(name="ps", bufs=3, space="PSUM") as psp,
              tc.tile_pool(name="pss", bufs=1, space="PSUM") as pss):
            # initial loads on two parallel HWDGE queues (sync + scalar),
            # ordered by first use so compute starts immediately.
            centT = [big.tile([128, 2, K], F32, name=f"centT{b}") for b in range(2)]
            cent_kd = [big.tile([128, 4, D], F32, name=f"centkd{b}") for b in range(2)]
            for dc in range(2):
                nc.sync.dma_start(centT[0][:, dc, :], c0t[dc * 128:(dc + 1) * 128, :])
            for kc in range(4):
                nc.sync.dma_start(cent_kd[0][:, kc, :], c0[kc * 128:(kc + 1) * 128, :])
            xt_sb = big.tile([128, 2, NSH], F32)
            for j in range(8):
                for dc in range(2):
                    nc.sync.dma_start(
                        xt_sb[:, dc, j * 1024:(j + 1) * 1024],
                        xt[dc * 128:(dc + 1) * 128, j * 1024:(j + 1) * 1024])
            xa_sb = [big.tile([128, NCH, DP], BF16, name=f"xa{s}_sb")
                     for s in range(3)]
            for i in range(NCH):
                for s in range(3):
                    nc.scalar.dma_start(xa_sb[s][:, i, :],
                                        xa3[s][i * 128:(i + 1) * 128, :])
            iota_sb = big.tile([128, K], F32)
            nc.scalar.dma_start(iota_sb[:, :], iotaf[:, :])
            ident_sb = big.tile([128, 128], F32)
            nc.scalar.dma_start(ident_sb[:, :], ident[:, :])
            # warm up the collective stack (full payload size) while the
            # input loads stream in. The send stages the last-loaded xa tile,
            # so this AR doubles as a cross-core load-completion barrier —
            # epoch 0's real AR then doesn't absorb inter-core load skew.
            warm_sb = small.tile([128, 2], F32, tag="warm")
            nc.gpsimd.tensor_copy(warm_sb[:, :], xa_sb[2][:, NCH - 1, 0:2])
            nc.sync.dma_start(warm_s[0:128, 0:2], warm_sb[:, :])
            nc.gpsimd.collective_compute(
                "AllReduce", OP.add, replica_groups=rg,
                ins=[warm_s[:, :].opt()], outs=[warm_r[:, :].opt()])

            def dist_stage(e, i, cur, last):
                # dist in quarter-K column blocks so epoch e's PE work can
                # begin as soon as the first updated centroid quarter lands.
                dist_ps = psp.tile([128, K], F32, tag="dist", name=f"dist_{e}_{i}")
                for q in range(2):
                    cols = slice(q * 256, (q + 1) * 256)
                    for dc in range(2):
                        nc.tensor.matmul(dist_ps[:, cols],
                                         xt_sb[:, dc, i * 128:(i + 1) * 128],
                                         cur[:, dc, cols],
                                         start=(dc == 0), stop=(dc == 1))
                minv = small.tile([128, 1], F32, tag="minv", name=f"minv_{e}_{i}")
                nc.vector.tensor_reduce(minv[:, :], dist_ps[:, :], axis=AX, op=OP.min)
                if not last:
                    A = work.tile([128, K], BF16, tag="A", name=f"A_{e}_{i}")
                    nc.vector.tensor_scalar(A[:, :], dist_ps[:, :], minv[:, :],
                                            None, OP.is_equal)
                    return A
                junk = work.tile([128, K], BF16, tag="junk", name=f"junk_{i}", bufs=2)
                idxf = small.tile([128, 1], F32, tag="idxf", name=f"idxf_{i}")
                nc.vector.scalar_tensor_tensor(junk[:, :], dist_ps[:, :],
                                               minv[:, :], iota_sb[:, :],
                                               OP.is_equal, OP.mult,
                                               accum_out=idxf[:, :])
                idxi = small.tile([128, 1], I32, tag="idxi", name=f"idxi_{i}")
                nc.vector.tensor_copy(idxi[:, :], idxf[:, :])
                nc.sync.dma_start(idx_out[i:i + 1, :], idxi[:, :])
                return None

            def sums_stage(i, A, sums_ps):
                for kc in range(4):
                    for s in range(3):
                        nc.tensor.matmul(sums_ps[kc][:, :],
                                         A[:, kc * 128:(kc + 1) * 128],
                                         xa_sb[s][:, i, :],
                                         start=(i == 0 and s == 0),
                                         stop=(i == NCH - 1 and s == 2))

            for t in range(trials):
              for e_ in range(EPOCHS):
                e = t * EPOCHS + e_
                last = e_ == EPOCHS - 1
                cur = centT[e_ % 2]
                sums_ps = None
                if not last:
                    sums_ps = [pss.tile([128, DP], F32, tag=f"sums{kc}",
                                        name=f"sums_{e}_{kc}") for kc in range(4)]
                pend = []
                for i in range(NCH):
                    A = dist_stage(e, i, cur, last)
                    if not last:
                        pend.append((i, A))
                        if len(pend) > SUMS_LAG:
                            j, Aj = pend.pop(0)
                            sums_stage(j, Aj, sums_ps)
                for j, Aj in pend:
                    sums_stage(j, Aj, sums_ps)
                if last:
                    continue

                ce = t * (EPOCHS - 1) + e_
                sums_sb = work.tile([128, 4, DP], F32, tag="sumssb",
                                    name=f"sumssb_{e}", bufs=1)
                sums_red = work.tile([128, 4, DP], F32, tag="sumsred",
                                     name=f"sumsred_{e}", bufs=1)
                for kc in range(4):
                    nc.vector.tensor_copy(sums_sb[:, kc, :], sums_ps[kc][:, :])
                    eng = nc.sync if kc % 2 == 0 else nc.scalar
                    eng.dma_start(snd[ce][kc * 128:(kc + 1) * 128, :],
                                  sums_sb[:, kc, :])
                nc.gpsimd.collective_compute(
                    "AllReduce", OP.add, replica_groups=rg,
                    ins=[snd[ce][:, :].opt()], outs=[rcv[ce][:, :].opt()])
                for kc in range(4):
                    eng = nc.sync if kc % 2 == 0 else nc.scalar
                    eng.dma_start(sums_red[:, kc, :],
                                  rcv[ce][kc * 128:(kc + 1) * 128, :])

                old_kd = cent_kd[e_ % 2]
                new_kd = cent_kd[(e_ + 1) % 2]
                nxt = centT[(e_ + 1) % 2]
                for kc in range(4):
                    counts = sums_red[:, kc, D:D + 1]
                    safe = small.tile([128, 1], F32, tag="safe", name=f"safe_{e}_{kc}")
                    nc.vector.tensor_scalar(safe[:, :], counts, 1.0, None, OP.max)
                    inv = small.tile([128, 1], F32, tag="inv", name=f"inv_{e}_{kc}")
                    nc.vector.reciprocal(inv[:, :], safe[:, :])
                    mask0 = small.tile([128, 1], F32, tag="mask0", name=f"m0_{e}_{kc}")
                    nc.vector.tensor_scalar(mask0[:, :], counts, 0.0, None, OP.is_equal)
                    oldm = work.tile([128, D], F32, tag="oldm", name=f"oldm_{e}_{kc}",
                                     bufs=2)
                    nc.vector.tensor_scalar(oldm[:, :], old_kd[:, kc, :],
                                            mask0[:, :], None, OP.mult)
                    # new = sums*inv + old*(counts==0), fused on DVE
                    nc.vector.scalar_tensor_tensor(new_kd[:, kc, :],
                                                   sums_red[:, kc, 0:D],
                                                   inv[:, :], oldm[:, :],
                                                   OP.mult, OP.add)
                    for dc in range(2):
                        tp = psp.tile([128, 128], F32, tag="dist",
                                      name=f"tp_{e}_{kc}_{dc}")
                        nc.tensor.transpose(tp[:, :],
                                            new_kd[:, kc, dc * 128:(dc + 1) * 128],
                                            ident_sb[:, :])
                        nc.vector.tensor_copy(nxt[:, dc, kc * 128:(kc + 1) * 128],
                                              tp[:, :])
    nc.compile()
    return nc


_NC_CACHE = {}


def get_nc(trials=1):
    if trials not in _NC_CACHE:
        _NC_CACHE[trials] = build(trials)
    return _NC_CACHE[trials]


def make_in_maps(x):
    bf = ml_dtypes.bfloat16
    x = np.ascontiguousarray(np.asarray(x, dtype=np.float32))
    assert x.shape == (N, D)
    cent0 = x[:K]
    c0t_np = np.ascontiguousarray(cent0.T)
    c0_np = np.ascontiguousarray(cent0)
    iota_np = np.broadcast_to(np.arange(K, dtype=np.float32), (128, K)).copy()
    ident_np = np.eye(128, dtype=np.float32)
    in_maps = []
    for r in range(N_CORES):
        xs = x[r * NSH:(r + 1) * NSH]
        xa_np = np.concatenate([xs, np.ones((NSH, 1), np.float32),
                                np.zeros((NSH, 1), np.float32)], axis=1)
        # exact 3-way bf16 split: xa == xh + xm + xl bit-exactly
        xh = xa_np.astype(bf)
        rr = xa_np - xh.astype(np.float32)
        xm = rr.astype(bf)
        xl = (rr - xm.astype(np.float32)).astype(bf)
        xt_np = np.ascontiguousarray(xs.T)
        in_maps.append({
            "xa0": np.ascontiguousarray(xh),
            "xa1": np.ascontiguousarray(xm),
            "xa2": np.ascontiguousarray(xl),
            "xt": xt_np,
            "c0t": c0t_np,
            "c0": c0_np,
            "iotaf": iota_np,
            "ident": ident_np,
        })
    return in_maps


def kernel(x):
    """Full-input k-means kernel: shards x over 8 TRN2 cores internally."""
    nc = get_nc()
    in_maps = make_in_maps(x)
    res = bass_utils.run_bass_kernel_spmd(nc, in_maps,
                                          core_ids=list(range(N_CORES)))
    idx = np.concatenate([res.results[r]["idx_out"].reshape(-1)
                          for r in range(N_CORES)]).astype(np.int32)
    return idx


# revision 29
# speedup vs baseline: 1.0464x; 1.0065x over previous
"""KMeans cluster kernel for 8-core TRN2 — builder + host wrapper.

Data-parallel over samples: each of the 8 cores owns 8192 rows of x.
Per epoch: dist = x @ cent.T via PE (fp32, exact), argmin via DVE
min-reduce + is_equal one-hot (written as bf16 — 0/1 exact),
per-centroid sums+counts via three bf16 one-hot matmuls accumulated in
PSUM (xa is split exactly into three bf16 terms hi/mid/lo on the host:
24-bit mantissa = 3x8-bit chunks, zero residual; bf16 matmuls stream
1 col/cycle vs fp32's 4), AllReduce across cores, centroid mean update
+ PE transpose. Epoch 10 extracts indices only (scalar_tensor_tensor
accum trick). Sums stage trails the dist stage by SUMS_LAG chunks so
the in-order PE never stalls on the DVE argmin chain.
"""

import numpy as np
import ml_dtypes
import concourse.bass as bass
import concourse.bacc as bacc
import concourse.tile as tile
import concourse.mybir as mybir
from concourse import bass_utils

N_CORES = 8
N = 65536
D = 256
DP = D + 2                # ones col (counts) + zero pad
K = 512
NSH = N // N_CORES        # rows per core
NCH = NSH // 128          # chunks of 128 rows
EPOCHS = 10

F32 = mybir.dt.float32
BF16 = mybir.dt.bfloat16
I32 = mybir.dt.int32
AX = mybir.AxisListType.X
OP = mybir.AluOpType

SUMS_LAG = 2              # chunks the sums stage trails the dist stage


def build(trials=1):
    nc = bacc.Bacc("TRN2", target_bir_lowering=False, debug=False,
                   num_devices=N_CORES)
    xa3 = [nc.dram_tensor(f"xa{s}", [NSH, DP], BF16, kind="ExternalInput").ap()
           for s in range(3)]
    xt = nc.dram_tensor("xt", [D, NSH], F32, kind="ExternalInput").ap()
    c0t = nc.dram_tensor("c0t", [D, K], F32, kind="ExternalInput").ap()
    c0 = nc.dram_tensor("c0", [K, D], F32, kind="ExternalInput").ap()
    iotaf = nc.dram_tensor("iotaf", [128, K], F32, kind="ExternalInput").ap()
    ident = nc.dram_tensor("ident", [128, 128], F32, kind="ExternalInput").ap()
    idx_out = nc.dram_tensor("idx_out", [NCH, 128], I32, kind="ExternalOutput").ap()

    snd = [nc.dram_tensor(f"snd{e}", [K, DP], F32, kind="Internal").ap()
           for e in range((EPOCHS - 1) * trials)]
    rcv = [nc.dram_tensor(f"rcv{e}", [K, DP], F32, kind="Internal",
                          addr_space="Shared").ap()
           for e in range((EPOCHS - 1) * trials)]
    warm_s = nc.dram_tensor("warm_s", [K, DP], F32, kind="Internal").ap()
    warm_r = nc.dram_tensor("warm_r", [K, DP], F32, kind="Internal",
                            addr_space="Shared").ap()
    rg = [list(range(N_CORES))]

    with tile.TileContext(nc) as tc:
        with (tc.tile_pool(name="big", bufs=1) as big,
              tc.tile_pool(name="work", bufs=3) as work,
              tc.tile_pool(name="small", bufs=8) as small,
              tc.tile_pool(name="ps", bufs=4, space="PSUM") as psp,
              tc.tile_pool(name="pss", bufs=1, space="PSUM") as pss):
            # initial loads on two parallel HWDGE queues (sync + scalar),
            # ordered by first use so compute starts immediately.
            centT = [big.tile([128, 2, K], F32, name=f"centT{b}") for b in range(2)]
            cent_kd = [big.tile([128, 4, D], F32, name=f"centkd{b}") for b in range(2)]
            for dc in range(2):
                nc.sync.dma_start(centT[0][:, dc, :], c0t[dc * 128:(dc + 1) * 128, :])
            for kc in range(4):
                nc.sync.dma_start(cent_kd[0][:, kc, :], c0[kc * 128:(kc + 1) * 128, :])
            xt_sb = big.tile([128, 2, NSH], F32)
            for j in range(8):
                for dc in range(2):
                    nc.sync.dma_start(
                        xt_sb[:, dc, j * 1024:(j + 1) * 1024],
                        xt[dc * 128:(dc + 1) * 128, j * 1024:(j + 1) * 1024])
            xa_sb = [big.tile([128, NCH, DP], BF16, name=f"xa{s}_sb")
                     for s in range(3)]
            for i in range(NCH):
                for s in range(3):
                    nc.scalar.dma_start(xa_sb[s][:, i, :],
                                        xa3[s][i * 128:(i + 1) * 128, :])
            iota_sb = big.tile([128, K], F32)
            nc.scalar.dma_start(iota_sb[:, :], iotaf[:, :])
            ident_sb = big.tile([128, 128], F32)
            nc.scalar.dma_start(ident_sb[:, :], ident[:, :])
            # warm up the collective stack (full payload size) while the
            # input loads stream in. The send stages the last-loaded xa tile,
            # so this AR doubles as a cross-core load-completion barrier —
            # epoch 0's real AR then doesn't absorb inter-core load skew.
            warm_sb = small.tile([128, 2], F32, tag="warm")
            nc.gpsimd.tensor_copy(warm_sb[:, :], xa_sb[2][:, NCH - 1, 0:2])
            nc.sync.dma_start(warm_s[0:128, 0:2], warm_sb[:, :])
            nc.gpsimd.collective_compute(
                "AllReduce", OP.add, replica_groups=rg,
                ins=[warm_s[:, :].opt()], outs=[warm_r[:, :].opt()])

            def dist_stage(e, i, cur, last):
                # dist in quarter-K column blocks so epoch e's PE work can
                # begin as soon as the first updated centroid quarter lands.
                dist_ps = psp.tile([128, K], F32, tag="dist", name=f"dist_{e}_{i}")
                for q in range(2):
                    cols = slice(q * 256, (q + 1) * 256)
                    for dc in range(2):
                        nc.tensor.matmul(dist_ps[:, cols],
                                         xt_sb[:, dc, i * 128:(i + 1) * 128],
                                         cur[:, dc, cols],
                                         start=(dc == 0), stop=(dc == 1))
                minv = small.tile([128, 1], F32, tag="minv", name=f"minv_{e}_{i}")
                nc.vector.tensor_reduce(minv[:, :], dist_ps[:, :], axis=AX, op=OP.min)
                if not last:
                    A = work.tile([128, K], BF16, tag="A", name=f"A_{e}_{i}")
                    nc.vector.tensor_scalar(A[:, :], dist_ps[:, :], minv[:, :],
                                            None, OP.is_equal)
                    return A
                junk = work.tile([128, K], BF16, tag="junk", name=f"junk_{i}", bufs=2)
                idxf = small.tile([128, 1], F32, tag="idxf", name=f"idxf_{i}")
                nc.vector.scalar_tensor_tensor(junk[:, :], dist_ps[:, :],
                                               minv[:, :], iota_sb[:, :],
                                               OP.is_equal, OP.mult,
                                               accum_out=idxf[:, :])
                idxi = small.tile([128, 1], I32, tag="idxi", name=f"idxi_{i}")
                nc.vector.tensor_copy(idxi[:, :], idxf[:, :])
                nc.sync.dma_start(idx_out[i:i + 1, :], idxi[:, :])
                return None

            def sums_stage(i, A, sums_ps):
                for kc in range(4):
                    for s in range(3):
                        nc.tensor.matmul(sums_ps[kc][:, :],
                                         A[:, kc * 128:(kc + 1) * 128],
                                         xa_sb[s][:, i, :],
                                         start=(i == 0 and s == 0),
                                         stop=(i == NCH - 1 and s == 2))

            for t in range(trials):
              for e_ in range(EPOCHS):
                e = t * EPOCHS + e_
                last = e_ == EPOCHS - 1
                cur = centT[e_ % 2]
                sums_ps = None
                if not last:
                    sums_ps = [pss.tile([128, DP], F32, tag=f"sums{kc}",
                                        name=f"sums_{e}_{kc}") for kc in range(4)]
                pend = []
                for i in range(NCH):
                    A = dist_stage(e, i, cur, last)
                    if not last:
                        pend.append((i, A))
                        if len(pend) > SUMS_LAG:
                            j, Aj = pend.pop(0)
                            sums_stage(j, Aj, sums_ps)
                for j, Aj in pend:
                    sums_stage(j, Aj, sums_ps)
                if last:
                    continue

                ce = t * (EPOCHS - 1) + e_
                sums_sb = work.tile([128, 4, DP], F32, tag="sumssb",
                                    name=f"sumssb_{e}", bufs=1)
                sums_red = work.tile([128, 4, DP], F32, tag="sumsred",
                                     name=f"sumsred_{e}", bufs=1)
                for kc in range(4):
                    nc.vector.tensor_copy(sums_sb[:, kc, :], sums_ps[kc][:, :])
                    eng = nc.sync if kc % 2 == 0 else nc.scalar
                    eng.dma_start(snd[ce][kc * 128:(kc + 1) * 128, :],
                                  sums_sb[:, kc, :])
                nc.gpsimd.collective_compute(
                    "AllReduce", OP.add, replica_groups=rg,
                    ins=[snd[ce][:, :].opt()], outs=[rcv[ce][:, :].opt()])
                for kc in range(4):
                    eng = nc.sync if kc % 2 == 0 else nc.scalar
                    eng.dma_start(sums_red[:, kc, :],
                                  rcv[ce][kc * 128:(kc + 1) * 128, :])

                old_kd = cent_kd[e_ % 2]
                new_kd = cent_kd[(e_ + 1) % 2]
                nxt = centT[(e_ + 1) % 2]
                for kc in range(4):
                    counts = sums_red[:, kc, D:D + 1]
                    safe = small.tile([128, 1], F32, tag="safe", name=f"safe_{e}_{kc}")
                    nc.vector.tensor_scalar(safe[:, :], counts, 1.0, None, OP.max)
                    inv = small.tile([128, 1], F32, tag="inv", name=f"inv_{e}_{kc}")
                    nc.vector.reciprocal(inv[:, :], safe[:, :])
                    mask0 = small.tile([128, 1], F32, tag="mask0", name=f"m0_{e}_{kc}")
                    nc.vector.tensor_scalar(mask0[:, :], counts, 0.0, None, OP.is_equal)
                    oldm = work.tile([128, D], F32, tag="oldm", name=f"oldm_{e}_{kc}",
                                     bufs=2)
                    nc.vector.tensor_scalar(oldm[:, :], old_kd[:, kc, :],
                                            mask0[:, :], None, OP.mult)
                    # new = sums*inv + old*(counts==0), fused on DVE
                    nc.vector.scalar_tensor_tensor(new_kd[:, kc, :],
                                                   sums_red[:, kc, 0:D],
                                                   inv[:, :], oldm[:, :],
                                                   OP.mult, OP.add)
                    for dc in range(2):
                        tp = psp.tile([128, 128], F32, tag="dist",
                                      name=f"tp_{e}_{kc}_{dc}")
                        nc.tensor.transpose(tp[:, :],
                                            new_kd[:, kc, dc * 128:(dc + 1) * 128],
                                            ident_sb[:, :])
                        nc.vector.tensor_copy(nxt[:, dc, kc * 128:(kc + 1) * 128],
                                              tp[:, :])
    nc.compile()
    return nc


_NC_CACHE = {}


def get_nc(trials=1):
    if trials not in _NC_CACHE:
        _NC_CACHE[trials] = build(trials)
    return _NC_CACHE[trials]


def make_in_maps(x):
    bf = ml_dtypes.bfloat16
    x = np.ascontiguousarray(np.asarray(x, dtype=np.float32))
    assert x.shape == (N, D)
    cent0 = x[:K]
    c0t_np = np.ascontiguousarray(cent0.T)
    c0_np = np.ascontiguousarray(cent0)
    iota_np = np.broadcast_to(np.arange(K, dtype=np.float32), (128, K)).copy()
    ident_np = np.eye(128, dtype=np.float32)
    in_maps = []
    for r in range(N_CORES):
        xs = x[r * NSH:(r + 1) * NSH]
        xa_np = np.concatenate([xs, np.ones((NSH, 1), np.float32),
                                np.zeros((NSH, 1), np.float32)], axis=1)
        # exact 3-way bf16 split: xa == xh + xm + xl bit-exactly
        xh = xa_np.astype(bf)
        rr = xa_np - xh.astype(np.float32)
        xm = rr.astype(bf)
        xl = (rr - xm.astype(np.float32)).astype(bf)
        xt_np = np.ascontiguousarray(xs.T)
        in_maps.append({
            "xa0": np.ascontiguousarray(xh),
            "xa1": np.ascontiguousarray(xm),
            "xa2": np.ascontiguousarray(xl),
            "xt": xt_np,
            "c0t": c0t_np,
            "c0": c0_np,
            "iotaf": iota_np,
            "ident": ident_np,
        })
    return in_maps


def kernel(x):
    """Full-input k-means kernel: shards x over 8 TRN2 cores internally."""
    nc = get_nc()
    in_maps = make_in_maps(x)
    res = bass_utils.run_bass_kernel_spmd(nc, in_maps,
                                          core_ids=list(range(N_CORES)))
    idx = np.concatenate([res.results[r]["idx_out"].reshape(-1)
                          for r in range(N_CORES)]).astype(np.int32)
    return idx


# revision 30
# speedup vs baseline: 1.1891x; 1.1364x over previous
"""KMeans cluster kernel for 8-core TRN2 — builder + host wrapper.

Data-parallel over samples: each of the 8 cores owns 8192 rows of x.
Per epoch: dist = x @ cent.T via PE (fp32, exact), argmin via DVE
min-reduce + is_equal one-hot (written as fp16 — 0/1 exact),
per-centroid sums+counts via two fp16 one-hot matmuls accumulated in
PSUM (xa is split into two fp16 terms hi/lo on the host, residual
2^-22 — the TRN2 PE preserves fp16 denormals exactly, verified; fp16
matmuls stream 1 col/cycle vs fp32's 4), AllReduce across cores,
centroid mean update
+ PE transpose. Epoch 10 extracts indices only (scalar_tensor_tensor
accum trick). Sums stage trails the dist stage by SUMS_LAG chunks so
the in-order PE never stalls on the DVE argmin chain.
"""

import numpy as np
import concourse.bass as bass
import concourse.bacc as bacc
import concourse.tile as tile
import concourse.mybir as mybir
from concourse import bass_utils

N_CORES = 8
N = 65536
D = 256
DP = D + 2                # ones col (counts) + zero pad
K = 512
NSH = N // N_CORES        # rows per core
NCH = NSH // 128          # chunks of 128 rows
EPOCHS = 10

F32 = mybir.dt.float32
F16 = mybir.dt.float16
I32 = mybir.dt.int32
AX = mybir.AxisListType.X
OP = mybir.AluOpType

SUMS_LAG = 2              # chunks the sums stage trails the dist stage


def build(trials=1):
    nc = bacc.Bacc("TRN2", target_bir_lowering=False, debug=False,
                   num_devices=N_CORES)
    xa2 = [nc.dram_tensor(f"xa{s}", [NSH, DP], F16, kind="ExternalInput").ap()
           for s in range(2)]
    xt = nc.dram_tensor("xt", [D, NSH], F32, kind="ExternalInput").ap()
    c0t = nc.dram_tensor("c0t", [D, K], F32, kind="ExternalInput").ap()
    c0 = nc.dram_tensor("c0", [K, D], F32, kind="ExternalInput").ap()
    iotaf = nc.dram_tensor("iotaf", [128, K], F32, kind="ExternalInput").ap()
    ident = nc.dram_tensor("ident", [128, 128], F32, kind="ExternalInput").ap()
    idx_out = nc.dram_tensor("idx_out", [NCH, 128], I32, kind="ExternalOutput").ap()

    snd = [nc.dram_tensor(f"snd{e}", [K, DP], F32, kind="Internal").ap()
           for e in range((EPOCHS - 1) * trials)]
    rcv = [nc.dram_tensor(f"rcv{e}", [K, DP], F32, kind="Internal",
                          addr_space="Shared").ap()
           for e in range((EPOCHS - 1) * trials)]
    warm_s = nc.dram_tensor("warm_s", [K, DP], F32, kind="Internal").ap()
    warm_r = nc.dram_tensor("warm_r", [K, DP], F32, kind="Internal",
                            addr_space="Shared").ap()
    rg = [list(range(N_CORES))]

    with tile.TileContext(nc) as tc:
        with (tc.tile_pool(name="big", bufs=1) as big,
              tc.tile_pool(name="work", bufs=3) as work,
              tc.tile_pool(name="small", bufs=8) as small,
              tc.tile_pool(name="ps", bufs=4, space="PSUM") as psp,
              tc.tile_pool(name="pss", bufs=1, space="PSUM") as pss):
            # initial loads on two parallel HWDGE queues (sync + scalar),
            # ordered by first use so compute starts immediately.
            centT = [big.tile([128, 2, K], F32, name=f"centT{b}") for b in range(2)]
            cent_kd = [big.tile([128, 4, D], F32, name=f"centkd{b}") for b in range(2)]
            for dc in range(2):
                nc.sync.dma_start(centT[0][:, dc, :], c0t[dc * 128:(dc + 1) * 128, :])
            for kc in range(4):
                nc.sync.dma_start(cent_kd[0][:, kc, :], c0[kc * 128:(kc + 1) * 128, :])
            xt_sb = big.tile([128, 2, NSH], F32)
            for j in range(8):
                for dc in range(2):
                    nc.sync.dma_start(
                        xt_sb[:, dc, j * 1024:(j + 1) * 1024],
                        xt[dc * 128:(dc + 1) * 128, j * 1024:(j + 1) * 1024])
            xa_sb = [big.tile([128, NCH, DP], F16, name=f"xa{s}_sb")
                     for s in range(2)]
            for i in range(NCH):
                for s in range(2):
                    nc.scalar.dma_start(xa_sb[s][:, i, :],
                                        xa2[s][i * 128:(i + 1) * 128, :])
            iota_sb = big.tile([128, K], F32)
            nc.scalar.dma_start(iota_sb[:, :], iotaf[:, :])
            ident_sb = big.tile([128, 128], F32)
            nc.scalar.dma_start(ident_sb[:, :], ident[:, :])
            # warm up the collective stack (full payload size) while the
            # input loads stream in. The send stages the last-loaded xa tile,
            # so this AR doubles as a cross-core load-completion barrier —
            # epoch 0's real AR then doesn't absorb inter-core load skew.
            warm_sb = small.tile([128, 2], F32, tag="warm")
            nc.gpsimd.tensor_copy(warm_sb[:, :], xa_sb[1][:, NCH - 1, 0:2])
            nc.sync.dma_start(warm_s[0:128, 0:2], warm_sb[:, :])
            nc.gpsimd.collective_compute(
                "AllReduce", OP.add, replica_groups=rg,
                ins=[warm_s[:, :].opt()], outs=[warm_r[:, :].opt()])

            def dist_stage(e, i, cur, last):
                # dist in quarter-K column blocks so epoch e's PE work can
                # begin as soon as the first updated centroid quarter lands.
                dist_ps = psp.tile([128, K], F32, tag="dist", name=f"dist_{e}_{i}")
                for q in range(2):
                    cols = slice(q * 256, (q + 1) * 256)
                    for dc in range(2):
                        nc.tensor.matmul(dist_ps[:, cols],
                                         xt_sb[:, dc, i * 128:(i + 1) * 128],
                                         cur[:, dc, cols],
                                         start=(dc == 0), stop=(dc == 1))
                minv = small.tile([128, 1], F32, tag="minv", name=f"minv_{e}_{i}")
                nc.vector.tensor_reduce(minv[:, :], dist_ps[:, :], axis=AX, op=OP.min)
                if not last:
                    A = work.tile([128, K], F16, tag="A", name=f"A_{e}_{i}")
                    nc.vector.tensor_scalar(A[:, :], dist_ps[:, :], minv[:, :],
                                            None, OP.is_equal)
                    return A
                junk = work.tile([128, K], F16, tag="junk", name=f"junk_{i}", bufs=2)
                idxf = small.tile([128, 1], F32, tag="idxf", name=f"idxf_{i}")
                nc.vector.scalar_tensor_tensor(junk[:, :], dist_ps[:, :],
                                               minv[:, :], iota_sb[:, :],
                                               OP.is_equal, OP.mult,
                                               accum_out=idxf[:, :])
                idxi = small.tile([128, 1], I32, tag="idxi", name=f"idxi_{i}")
                nc.vector.tensor_copy(idxi[:, :], idxf[:, :])
                nc.sync.dma_start(idx_out[i:i + 1, :], idxi[:, :])
                return None

            def sums_stage(i, A, sums_ps):
                for kc in range(4):
                    for s in range(2):
                        nc.tensor.matmul(sums_ps[kc][:, :],
                                         A[:, kc * 128:(kc + 1) * 128],
                                         xa_sb[s][:, i, :],
                                         start=(i == 0 and s == 0),
                                         stop=(i == NCH - 1 and s == 1))

            for t in range(trials):
              for e_ in range(EPOCHS):
                e = t * EPOCHS + e_
                last = e_ == EPOCHS - 1
                cur = centT[e_ % 2]
                sums_ps = None
                if not last:
                    sums_ps = [pss.tile([128, DP], F32, tag=f"sums{kc}",
                                        name=f"sums_{e}_{kc}") for kc in range(4)]
                pend = []
                for i in range(NCH):
                    A = dist_stage(e, i, cur, last)
                    if not last:
                        pend.append((i, A))
                        if len(pend) > SUMS_LAG:
                            j, Aj = pend.pop(0)
                            sums_stage(j, Aj, sums_ps)
                for j, Aj in pend:
                    sums_stage(j, Aj, sums_ps)
                if last:
                    continue

                ce = t * (EPOCHS - 1) + e_
                sums_sb = work.tile([128, 4, DP], F32, tag="sumssb",
                                    name=f"sumssb_{e}", bufs=1)
                sums_red = work.tile([128, 4, DP], F32, tag="sumsred",
                                     name=f"sumsred_{e}", bufs=1)
                for kc in range(4):
                    nc.vector.tensor_copy(sums_sb[:, kc, :], sums_ps[kc][:, :])
                    eng = nc.sync if kc % 2 == 0 else nc.scalar
                    eng.dma_start(snd[ce][kc * 128:(kc + 1) * 128, :],
                                  sums_sb[:, kc, :])
                nc.gpsimd.collective_compute(
                    "AllReduce", OP.add, replica_groups=rg,
                    ins=[snd[ce][:, :].opt()], outs=[rcv[ce][:, :].opt()])
                for kc in range(4):
                    eng = nc.sync if kc % 2 == 0 else nc.scalar
                    eng.dma_start(sums_red[:, kc, :],
                                  rcv[ce][kc * 128:(kc + 1) * 128, :])

                old_kd = cent_kd[e_ % 2]
                new_kd = cent_kd[(e_ + 1) % 2]
                nxt = centT[(e_ + 1) % 2]
                for kc in range(4):
                    counts = sums_red[:, kc, D:D + 1]
                    safe = small.tile([128, 1], F32, tag="safe", name=f"safe_{e}_{kc}")
                    nc.vector.tensor_scalar(safe[:, :], counts, 1.0, None, OP.max)
                    inv = small.tile([128, 1], F32, tag="inv", name=f"inv_{e}_{kc}")
                    nc.vector.reciprocal(inv[:, :], safe[:, :])
                    mask0 = small.tile([128, 1], F32, tag="mask0", name=f"m0_{e}_{kc}")
                    nc.vector.tensor_scalar(mask0[:, :], counts, 0.0, None, OP.is_equal)
                    oldm = work.tile([128, D], F32, tag="oldm", name=f"oldm_{e}_{kc}",
                                     bufs=2)
                    nc.vector.tensor_scalar(oldm[:, :], old_kd[:, kc, :],
                                            mask0[:, :], None, OP.mult)
                    # new = sums*inv + old*(counts==0), fused on DVE
                    nc.vector.scalar_tensor_tensor(new_kd[:, kc, :],
                                                   sums_red[:, kc, 0:D],
                                                   inv[:, :], oldm[:, :],
                                                   OP.mult, OP.add)
                    for dc in range(2):
                        tp = psp.tile([128, 128], F32, tag="dist",
                                      name=f"tp_{e}_{kc}_{dc}")
                        nc.tensor.transpose(tp[:, :],
                                            new_kd[:, kc, dc * 128:(dc + 1) * 128],
                                            ident_sb[:, :])
                        nc.vector.tensor_copy(nxt[:, dc, kc * 128:(kc + 1) * 128],
                                              tp[:, :])
    nc.compile()
    return nc


_NC_CACHE = {}


def get_nc(trials=1):
    if trials not in _NC_CACHE:
        _NC_CACHE[trials] = build(trials)
    return _NC_CACHE[trials]


def make_in_maps(x):
    x = np.ascontiguousarray(np.asarray(x, dtype=np.float32))
    assert x.shape == (N, D)
    cent0 = x[:K]
    c0t_np = np.ascontiguousarray(cent0.T)
    c0_np = np.ascontiguousarray(cent0)
    iota_np = np.broadcast_to(np.arange(K, dtype=np.float32), (128, K)).copy()
    ident_np = np.eye(128, dtype=np.float32)
    in_maps = []
    for r in range(N_CORES):
        xs = x[r * NSH:(r + 1) * NSH]
        xa_np = np.concatenate([xs, np.ones((NSH, 1), np.float32),
                                np.zeros((NSH, 1), np.float32)], axis=1)
        # fp16 hi/lo split: xa ~ xh + xl with residual <= 2^-22 |xa|
        xh = xa_np.astype(np.float16)
        xl = (xa_np - xh.astype(np.float32)).astype(np.float16)
        xt_np = np.ascontiguousarray(xs.T)
        in_maps.append({
            "xa0": np.ascontiguousarray(xh),
            "xa1": np.ascontiguousarray(xl),
            "xt": xt_np,
            "c0t": c0t_np,
            "c0": c0_np,
            "iotaf": iota_np,
            "ident": ident_np,
        })
    return in_maps


def kernel(x):
    """Full-input k-means kernel: shards x over 8 TRN2 cores internally."""
    nc = get_nc()
    in_maps = make_in_maps(x)
    res = bass_utils.run_bass_kernel_spmd(nc, in_maps,
                                          core_ids=list(range(N_CORES)))
    idx = np.concatenate([res.results[r]["idx_out"].reshape(-1)
                          for r in range(N_CORES)]).astype(np.int32)
    return idx


# revision 31
# speedup vs baseline: 1.3666x; 1.1492x over previous
"""KMeans cluster kernel for 8-core TRN2 — builder + host wrapper.

Data-parallel over samples: each of the 8 cores owns 8192 rows of x.
Per epoch: dist = x @ cent.T via PE (fp32, exact), argmin via DVE
min-reduce + is_equal one-hot (written as fp16 — 0/1 exact),
per-centroid sums+counts via two fp16 one-hot matmuls accumulated in
PSUM (xa is split into two fp16 terms hi/lo on the host, residual
2^-22 — the TRN2 PE preserves fp16 denormals exactly, verified; fp16
matmuls stream 1 col/cycle vs fp32's 4), AllReduce across cores,
centroid mean update
+ PE transpose. Epoch 10 extracts indices only (scalar_tensor_tensor
accum trick). Sums stage trails the dist stage by SUMS_LAG chunks so
the in-order PE never stalls on the DVE argmin chain.
"""

import numpy as np
import concourse.bass as bass
import concourse.bacc as bacc
import concourse.tile as tile
import concourse.mybir as mybir
from concourse import bass_utils

N_CORES = 8
N = 65536
D = 256
DP = D + 2                # ones col (counts) + zero pad
K = 512
NSH = N // N_CORES        # rows per core
NCH = NSH // 128          # chunks of 128 rows
EPOCHS = 10

F32 = mybir.dt.float32
F16 = mybir.dt.float16
I32 = mybir.dt.int32
AX = mybir.AxisListType.X
OP = mybir.AluOpType

SUMS_LAG = 2              # chunks the sums stage trails the dist stage


def build(trials=1):
    nc = bacc.Bacc("TRN2", target_bir_lowering=False, debug=False,
                   num_devices=N_CORES)
    xa2 = [nc.dram_tensor(f"xa{s}", [NSH, DP], F16, kind="ExternalInput").ap()
           for s in range(2)]
    xt2 = [nc.dram_tensor(f"xt{s}", [D, NSH], F16, kind="ExternalInput").ap()
           for s in range(2)]
    c0t2 = [nc.dram_tensor(f"c0t{s}", [D, K], F16, kind="ExternalInput").ap()
            for s in range(2)]
    c0 = nc.dram_tensor("c0", [K, D], F32, kind="ExternalInput").ap()
    iotaf = nc.dram_tensor("iotaf", [128, K], F32, kind="ExternalInput").ap()
    ident = nc.dram_tensor("ident", [128, 128], F32, kind="ExternalInput").ap()
    idx_out = nc.dram_tensor("idx_out", [NCH, 128], I32, kind="ExternalOutput").ap()

    snd = [nc.dram_tensor(f"snd{e}", [K, DP], F32, kind="Internal").ap()
           for e in range((EPOCHS - 1) * trials)]
    rcv = [nc.dram_tensor(f"rcv{e}", [K, DP], F32, kind="Internal",
                          addr_space="Shared").ap()
           for e in range((EPOCHS - 1) * trials)]
    warm_s = nc.dram_tensor("warm_s", [K, DP], F32, kind="Internal").ap()
    warm_r = nc.dram_tensor("warm_r", [K, DP], F32, kind="Internal",
                            addr_space="Shared").ap()
    rg = [list(range(N_CORES))]

    with tile.TileContext(nc) as tc:
        with (tc.tile_pool(name="big", bufs=1) as big,
              tc.tile_pool(name="work", bufs=3) as work,
              tc.tile_pool(name="small", bufs=8) as small,
              tc.tile_pool(name="ps", bufs=4, space="PSUM") as psp,
              tc.tile_pool(name="pss", bufs=1, space="PSUM") as pss):
            # initial loads on two parallel HWDGE queues (sync + scalar),
            # ordered by first use so compute starts immediately.
            centT = [big.tile([128, 2, K], F32, name=f"centT{b}") for b in range(2)]
            cent_kd = [big.tile([128, 4, D], F32, name=f"centkd{b}") for b in range(2)]
            chT = [big.tile([128, 2, K], F16, name=f"chT{b}") for b in range(2)]
            clT = [big.tile([128, 2, K], F16, name=f"clT{b}") for b in range(2)]
            for dc in range(2):
                nc.sync.dma_start(chT[0][:, dc, :], c0t2[0][dc * 128:(dc + 1) * 128, :])
                nc.sync.dma_start(clT[0][:, dc, :], c0t2[1][dc * 128:(dc + 1) * 128, :])
            for kc in range(4):
                nc.sync.dma_start(cent_kd[0][:, kc, :], c0[kc * 128:(kc + 1) * 128, :])
            xt_sb = [big.tile([128, 2, NSH], F16, name=f"xt{s}_sb")
                     for s in range(2)]
            for j in range(8):
                for dc in range(2):
                    for s in range(2):
                        nc.sync.dma_start(
                            xt_sb[s][:, dc, j * 1024:(j + 1) * 1024],
                            xt2[s][dc * 128:(dc + 1) * 128, j * 1024:(j + 1) * 1024])
            xa_sb = [big.tile([128, NCH, DP], F16, name=f"xa{s}_sb")
                     for s in range(2)]
            for i in range(NCH):
                for s in range(2):
                    nc.scalar.dma_start(xa_sb[s][:, i, :],
                                        xa2[s][i * 128:(i + 1) * 128, :])
            iota_sb = big.tile([128, K], F32)
            nc.scalar.dma_start(iota_sb[:, :], iotaf[:, :])
            ident_sb = big.tile([128, 128], F32)
            nc.scalar.dma_start(ident_sb[:, :], ident[:, :])
            # warm up the collective stack (full payload size) while the
            # input loads stream in. The send stages the last-loaded xa tile,
            # so this AR doubles as a cross-core load-completion barrier —
            # epoch 0's real AR then doesn't absorb inter-core load skew.
            warm_sb = small.tile([128, 2], F32, tag="warm")
            nc.gpsimd.tensor_copy(warm_sb[:, :], xa_sb[1][:, NCH - 1, 0:2])
            nc.sync.dma_start(warm_s[0:128, 0:2], warm_sb[:, :])
            nc.gpsimd.collective_compute(
                "AllReduce", OP.add, replica_groups=rg,
                ins=[warm_s[:, :].opt()], outs=[warm_r[:, :].opt()])

            def dist_stage(e, i, ch, cl, last):
                # dist = (xh+xl)(ch+cl) dropping xl*cl (~2^-22): 6 fp16
                # matmuls at 1 cyc/col vs fp32's 4 cyc/col, in half-K blocks.
                dist_ps = psp.tile([128, K], F32, tag="dist", name=f"dist_{e}_{i}")
                rows = slice(i * 128, (i + 1) * 128)
                for q in range(2):
                    cols = slice(q * 256, (q + 1) * 256)
                    for dc in range(2):
                        first = dc == 0
                        last_mm = dc == 1
                        nc.tensor.matmul(dist_ps[:, cols],
                                         xt_sb[0][:, dc, rows],
                                         ch[:, dc, cols],
                                         start=first, stop=False)
                        nc.tensor.matmul(dist_ps[:, cols],
                                         xt_sb[0][:, dc, rows],
                                         cl[:, dc, cols],
                                         start=False, stop=False)
                        nc.tensor.matmul(dist_ps[:, cols],
                                         xt_sb[1][:, dc, rows],
                                         ch[:, dc, cols],
                                         start=False, stop=last_mm)
                minv = small.tile([128, 1], F32, tag="minv", name=f"minv_{e}_{i}")
                nc.vector.tensor_reduce(minv[:, :], dist_ps[:, :], axis=AX, op=OP.min)
                if not last:
                    A = work.tile([128, K], F16, tag="A", name=f"A_{e}_{i}")
                    nc.vector.tensor_scalar(A[:, :], dist_ps[:, :], minv[:, :],
                                            None, OP.is_equal)
                    return A
                junk = work.tile([128, K], F16, tag="junk", name=f"junk_{i}", bufs=2)
                idxf = small.tile([128, 1], F32, tag="idxf", name=f"idxf_{i}")
                nc.vector.scalar_tensor_tensor(junk[:, :], dist_ps[:, :],
                                               minv[:, :], iota_sb[:, :],
                                               OP.is_equal, OP.mult,
                                               accum_out=idxf[:, :])
                idxi = small.tile([128, 1], I32, tag="idxi", name=f"idxi_{i}")
                nc.vector.tensor_copy(idxi[:, :], idxf[:, :])
                nc.sync.dma_start(idx_out[i:i + 1, :], idxi[:, :])
                return None

            def sums_stage(i, A, sums_ps):
                for kc in range(4):
                    for s in range(2):
                        nc.tensor.matmul(sums_ps[kc][:, :],
                                         A[:, kc * 128:(kc + 1) * 128],
                                         xa_sb[s][:, i, :],
                                         start=(i == 0 and s == 0),
                                         stop=(i == NCH - 1 and s == 1))

            for t in range(trials):
              for e_ in range(EPOCHS):
                e = t * EPOCHS + e_
                last = e_ == EPOCHS - 1
                cur_ch = chT[e_ % 2]
                cur_cl = clT[e_ % 2]
                sums_ps = None
                if not last:
                    sums_ps = [pss.tile([128, DP], F32, tag=f"sums{kc}",
                                        name=f"sums_{e}_{kc}") for kc in range(4)]
                pend = []
                for i in range(NCH):
                    A = dist_stage(e, i, cur_ch, cur_cl, last)
                    if not last:
                        pend.append((i, A))
                        if len(pend) > SUMS_LAG:
                            j, Aj = pend.pop(0)
                            sums_stage(j, Aj, sums_ps)
                for j, Aj in pend:
                    sums_stage(j, Aj, sums_ps)
                if last:
                    continue

                ce = t * (EPOCHS - 1) + e_
                sums_sb = work.tile([128, 4, DP], F32, tag="sumssb",
                                    name=f"sumssb_{e}", bufs=1)
                sums_red = work.tile([128, 4, DP], F32, tag="sumsred",
                                     name=f"sumsred_{e}", bufs=1)
                for kc in range(4):
                    nc.vector.tensor_copy(sums_sb[:, kc, :], sums_ps[kc][:, :])
                    eng = nc.sync if kc % 2 == 0 else nc.scalar
                    eng.dma_start(snd[ce][kc * 128:(kc + 1) * 128, :],
                                  sums_sb[:, kc, :])
                nc.gpsimd.collective_compute(
                    "AllReduce", OP.add, replica_groups=rg,
                    ins=[snd[ce][:, :].opt()], outs=[rcv[ce][:, :].opt()])
                for kc in range(4):
                    eng = nc.sync if kc % 2 == 0 else nc.scalar
                    eng.dma_start(sums_red[:, kc, :],
                                  rcv[ce][kc * 128:(kc + 1) * 128, :])

                old_kd = cent_kd[e_ % 2]
                new_kd = cent_kd[(e_ + 1) % 2]
                nxt = centT[(e_ + 1) % 2]
                nxt_ch = chT[(e_ + 1) % 2]
                nxt_cl = clT[(e_ + 1) % 2]
                for kc in range(4):
                    counts = sums_red[:, kc, D:D + 1]
                    safe = small.tile([128, 1], F32, tag="safe", name=f"safe_{e}_{kc}")
                    nc.vector.tensor_scalar(safe[:, :], counts, 1.0, None, OP.max)
                    inv = small.tile([128, 1], F32, tag="inv", name=f"inv_{e}_{kc}")
                    nc.vector.reciprocal(inv[:, :], safe[:, :])
                    mask0 = small.tile([128, 1], F32, tag="mask0", name=f"m0_{e}_{kc}")
                    nc.vector.tensor_scalar(mask0[:, :], counts, 0.0, None, OP.is_equal)
                    oldm = work.tile([128, D], F32, tag="oldm", name=f"oldm_{e}_{kc}",
                                     bufs=2)
                    nc.vector.tensor_scalar(oldm[:, :], old_kd[:, kc, :],
                                            mask0[:, :], None, OP.mult)
                    # new = sums*inv + old*(counts==0), fused on DVE
                    nc.vector.scalar_tensor_tensor(new_kd[:, kc, :],
                                                   sums_red[:, kc, 0:D],
                                                   inv[:, :], oldm[:, :],
                                                   OP.mult, OP.add)
                    for dc in range(2):
                        tp = psp.tile([128, 128], F32, tag="dist",
                                      name=f"tp_{e}_{kc}_{dc}")
                        nc.tensor.transpose(tp[:, :],
                                            new_kd[:, kc, dc * 128:(dc + 1) * 128],
                                            ident_sb[:, :])
                        nc.vector.tensor_copy(nxt[:, dc, kc * 128:(kc + 1) * 128],
                                              tp[:, :])
                # fp16 hi/lo split of the new centroids for the next epoch's
                # dist matmuls: ch = f16(c); cl = f16(c - ch)
                chf = work.tile([128, 2, K], F32, tag="chf", name=f"chf_{e}",
                                bufs=1)
                for dc in range(2):
                    nc.vector.tensor_copy(nxt_ch[:, dc, :], nxt[:, dc, :])
                    nc.vector.tensor_copy(chf[:, dc, :], nxt_ch[:, dc, :])
                    nc.vector.tensor_tensor(nxt_cl[:, dc, :], nxt[:, dc, :],
                                            chf[:, dc, :], OP.subtract)
    nc.compile()
    return nc


_NC_CACHE = {}


def get_nc(trials=1):
    if trials not in _NC_CACHE:
        _NC_CACHE[trials] = build(trials)
    return _NC_CACHE[trials]


def make_in_maps(x):
    x = np.ascontiguousarray(np.asarray(x, dtype=np.float32))
    assert x.shape == (N, D)
    cent0 = x[:K]
    c0t_np = np.ascontiguousarray(cent0.T)
    c0t2h_np = np.ascontiguousarray(c0t_np.astype(np.float16))
    c0t2l_np = np.ascontiguousarray(
        (c0t_np - c0t2h_np.astype(np.float32)).astype(np.float16))
    c0_np = np.ascontiguousarray(cent0)
    iota_np = np.broadcast_to(np.arange(K, dtype=np.float32), (128, K)).copy()
    ident_np = np.eye(128, dtype=np.float32)
    in_maps = []
    for r in range(N_CORES):
        xs = x[r * NSH:(r + 1) * NSH]
        xa_np = np.concatenate([xs, np.ones((NSH, 1), np.float32),
                                np.zeros((NSH, 1), np.float32)], axis=1)
        # fp16 hi/lo split: xa ~ xh + xl with residual <= 2^-22 |xa|
        xh = xa_np.astype(np.float16)
        xl = (xa_np - xh.astype(np.float32)).astype(np.float16)
        xt_np = np.ascontiguousarray(xs.T)
        xth = xt_np.astype(np.float16)
        xtl = (xt_np - xth.astype(np.float32)).astype(np.float16)
        in_maps.append({
            "xa0": np.ascontiguousarray(xh),
            "xa1": np.ascontiguousarray(xl),
            "xt0": np.ascontiguousarray(xth),
            "xt1": np.ascontiguousarray(xtl),
            "c0t0": c0t2h_np,
            "c0t1": c0t2l_np,
            "c0": c0_np,
            "iotaf": iota_np,
            "ident": ident_np,
        })
    return in_maps


def kernel(x):
    """Full-input k-means kernel: shards x over 8 TRN2 cores internally."""
    nc = get_nc()
    in_maps = make_in_maps(x)
    res = bass_utils.run_bass_kernel_spmd(nc, in_maps,
                                          core_ids=list(range(N_CORES)))
    idx = np.concatenate([res.results[r]["idx_out"].reshape(-1)
                          for r in range(N_CORES)]).astype(np.int32)
    return idx
